# revision 1
# baseline (speedup 1.0000x reference)
"""Trainium2 Bass kernel for nn_Meta_Graph1_40114994545303 (gnn_message_passing).

Math: the reference returns only the global-node row of the GCN output.
With mask = (attribute_label > 0), star adjacency means
    out[s, :] = tanh( (sum_a mask[s,a] * attribute_feat[s,a,:]) @ W + b )
and x never reaches the output (adj[A, A] = 0).

Strategy: data-parallel over batch, 32 samples per core on 8 cores.
Per core:
  stage 1: masked sum over attributes as a block-diagonal matmul
           (feat streamed as the moving operand, mask block-diag stationary)
  transpose the [32, 2048] masked sum to [2048, 32] via DVE 32x32 blocks
  stage 2: [32, 2048] @ W as 16 K-chunk matmuls with the masked-sum
           transposed chunks stationary and W streamed; bias folded in as a
           rank-1 (K=1) matmul of ones x b into the same PSUM accumulation
  tanh on the scalar engine, DMA out.
"""

import os

import numpy as np

import concourse.bacc as bacc
import concourse.mybir as mybir
from concourse.tile import TileContext

B, A, D = 256, 32, 2048
NCORES = 8
S = B // NCORES  # 32 samples per core
P = 128
KC1 = (S * A) // P  # 8 k-chunks in stage 1 (contraction over (sample, attr))
KC2 = D // P  # 16 k-chunks in stage 2 (contraction over d_in)
NT = D // 512  # 4 psum-bank-wide column tiles
F32 = mybir.dt.float32

COMPUTE_DTYPE = os.environ.get("GNN_KERNEL_DTYPE", "fp16")


def build_nc(compute_dtype: str = COMPUTE_DTYPE):
    cdt = {"f32": mybir.dt.float32, "bf16": mybir.dt.bfloat16, "fp16": mybir.dt.float16}[compute_dtype]
    # DMA batching: k-chunks (128 rows x 2048 cols) per dma_start.
    cf = 2 if compute_dtype == "f32" else 4  # feat chunks per DMA
    cw = 4 if compute_dtype == "f32" else 4  # W chunks per DMA
    wbufs = 2 if compute_dtype == "f32" else 4
    nc = bacc.Bacc("TRN2", target_bir_lowering=False, debug=False)

    feat = nc.dram_tensor("feat", [S * A, D], cdt, kind="ExternalInput")
    mbdt = nc.dram_tensor("mbdt", [P, KC1 * S], cdt, kind="ExternalInput")
    w = nc.dram_tensor("w", [D, D], cdt, kind="ExternalInput")
    bias = nc.dram_tensor("bias", [1, D], cdt, kind="ExternalInput")
    out = nc.dram_tensor("out", [S, D], F32, kind="ExternalOutput")

    with TileContext(nc) as tc:
        with (
            tc.tile_pool(name="const", bufs=1) as cpool,
            tc.tile_pool(name="featp", bufs=2) as fpool,
            tc.tile_pool(name="wp", bufs=wbufs) as wpool,
            tc.tile_pool(name="msc", bufs=1) as mcpool,
            tc.tile_pool(name="mst", bufs=1) as mpool,
            tc.tile_pool(name="outp", bufs=1) as opool,
            tc.tile_pool(name="ps1", bufs=1, space="PSUM") as ps1,
            tc.tile_pool(name="ps2", bufs=1, space="PSUM") as ps2,
        ):
            # constants (on the scalar HWDGE queue so the sync queue starts
            # streaming feat/W immediately)
            mbdt_t = cpool.tile([P, KC1, S], cdt)
            nc.scalar.dma_start(mbdt_t[:], mbdt[:].rearrange("p (k j) -> p k j", k=KC1))
            ones_t = cpool.tile([1, S], cdt)
            nc.vector.memset(ones_t[:], 1.0)
            bias_t = cpool.tile([1, D], cdt)
            nc.scalar.dma_start(bias_t[:], bias[:])

            msT = mpool.tile([P, KC2, S], cdt)  # masked_sum transposed

            # Column-group tiling: the four 512-wide output slices live at
            # partition offsets 0/32/64/96 of ONE psum bank, so their four
            # matmuls (same stationary operand) run concurrently in four PE
            # column groups instead of serializing on the weight reload.
            pm_bank = ps1.tile([P, 512], F32)
            po_bank = ps2.tile([P, 512], F32)

            # ---- stage 1: masked_sum[j, d] = sum_(s,a) mbd[(s,a), j] feat[(s,a), d]
            for g in range(KC1 // cf):
                ft = fpool.tile([P, cf, D], cdt)
                nc.sync.dma_start(
                    ft[:],
                    feat[g * cf * P : (g + 1) * cf * P, :].rearrange(
                        "(c p) d -> p c d", p=P
                    ),
                )
                for c in range(cf):
                    k = g * cf + c
                    for n in range(NT):
                        nc.tensor.matmul(
                            pm_bank[n * S : (n + 1) * S, :],
                            mbdt_t[:, k, :],
                            ft[:, c, n * 512 : (n + 1) * 512],
                            start=(k == 0),
                            stop=(k == KC1 - 1),
                            tile_position=(0, n * S),
                            skip_group_check=True,
                        )
            # cast/copy psum -> sbuf, then 32x32 block transposes into msT
            msc = mcpool.tile([P, 512], cdt)
            nc.any.tensor_copy(msc[:], pm_bank[:])
            for n in range(NT):
                for q in range(512 // 32):
                    d0 = n * 512 + q * 32
                    k2, r = divmod(d0, P)
                    nc.vector.transpose(
                        msT[r : r + 32, k2, :],
                        msc[n * S : (n + 1) * S, q * 32 : (q + 1) * 32],
                    )

            # ---- stage 2: out = tanh(masked_sum @ W + b)
            for g in range(KC2 // cw):
                wt = wpool.tile([P, cw, D], cdt)
                nc.sync.dma_start(
                    wt[:],
                    w[g * cw * P : (g + 1) * cw * P, :].rearrange(
                        "(c p) d -> p c d", p=P
                    ),
                )
                for c in range(cw):
                    k2 = g * cw + c
                    for n in range(NT):
                        nc.tensor.matmul(
                            po_bank[n * S : (n + 1) * S, :],
                            msT[:, k2, :],
                            wt[:, c, n * 512 : (n + 1) * 512],
                            start=(k2 == 0),
                            stop=False,
                            tile_position=(0, n * S),
                            skip_group_check=True,
                        )
            for n in range(NT):
                # bias as rank-1 matmul: po[j, :] += ones[j] * b[:]
                nc.tensor.matmul(
                    po_bank[n * S : (n + 1) * S, :],
                    ones_t[:],
                    bias_t[:, n * 512 : (n + 1) * 512],
                    start=False,
                    stop=True,
                    tile_position=(0, n * S),
                    skip_group_check=True,
                )
            out_sb = opool.tile([P, 512], F32)
            nc.scalar.activation(
                out_sb[:], po_bank[:], mybir.ActivationFunctionType.Tanh
            ).then_inc(act_sem, 1)
            scalar.wait_ge(act_sem, 1)
            for n in range(NT):
                nc.scalar.dma_start(
                    out[:, n * 512 : (n + 1) * 512],
                    out_sb[n * S : (n + 1) * S, :],
                )
    nc.compile()
    return nc


def build_nc_raw(compute_dtype: str = COMPUTE_DTYPE):
    """Raw-bass (no Tile) version: manual semaphores, everything resident in
    SBUF (feat 32KB/part + W 64KB/part at fp16), minimal prologue/epilogue."""
    cdt = {"f32": mybir.dt.float32, "bf16": mybir.dt.bfloat16, "fp16": mybir.dt.float16}[compute_dtype]
    cf = 4
    WCH = [4, 4, 4, 4]  # uniform W transfer sizes (k2-chunks)
    WST = [0, 4, 8, 12]
    NF, NW = KC1 // cf, len(WCH)
    nc = bacc.Bacc("TRN2", target_bir_lowering=False, debug=False)

    feat = nc.dram_tensor("feat", [S * A, D], cdt, kind="ExternalInput")
    mbdt = nc.dram_tensor("mbdt", [P, KC1 * S], cdt, kind="ExternalInput")
    w = nc.dram_tensor("w", [D, D], cdt, kind="ExternalInput")
    bias = nc.dram_tensor("bias", [1, D], cdt, kind="ExternalInput")
    onesd = nc.dram_tensor("ones", [1, S], cdt, kind="ExternalInput")
    out = nc.dram_tensor("out", [S, D], F32, kind="ExternalOutput")

    from contextlib import ExitStack

    with ExitStack() as ctx:
        feat_sb = ctx.enter_context(nc.sbuf_tensor([P, KC1, D], cdt))
        w_sb = ctx.enter_context(nc.sbuf_tensor([P, KC2, D], cdt))
        mbdt_sb = ctx.enter_context(nc.sbuf_tensor([P, KC1, S], cdt))
        bias_sb = ctx.enter_context(nc.sbuf_tensor([1, D], cdt))
        ones_sb = ctx.enter_context(nc.sbuf_tensor([1, S], cdt))
        msc_sb = ctx.enter_context(nc.sbuf_tensor([P, 512], cdt))
        msT_sb = ctx.enter_context(nc.sbuf_tensor([P, KC2, S], cdt))
        out_sb = ctx.enter_context(nc.sbuf_tensor([P, 512], F32))
        pm_bank = ctx.enter_context(nc.psum_tensor([P, 512], F32))
        po_bank = ctx.enter_context(nc.psum_tensor([P, 512], F32))
        fsems = [ctx.enter_context(nc.semaphore(f"fs{g}")) for g in range(NF)]
        wsems = [ctx.enter_context(nc.semaphore(f"ws{g}")) for g in range(NW)]
        csem = ctx.enter_context(nc.semaphore("csem"))
        osem = ctx.enter_context(nc.semaphore("osem"))
        s1_sem = ctx.enter_context(nc.semaphore("s1_sem"))
        tr_sem = ctx.enter_context(nc.semaphore("tr_sem"))
        s2_sem = ctx.enter_context(nc.semaphore("s2_sem"))
        act_sem = ctx.enter_context(nc.semaphore("act_sem"))
        osem2 = ctx.enter_context(nc.semaphore("osem2"))
        block = ctx.enter_context(nc.Block(no_gpsimd_drain=True))

        @block.sync
        def _(sync):
            for g in range(NF):
                sync.dma_start(
                    feat_sb[:, g * cf : (g + 1) * cf, :],
                    feat[g * cf * P : (g + 1) * cf * P, :].rearrange(
                        "(c p) d -> p c d", p=P
                    ),
                ).then_inc(fsems[g], 16)
            for g in range(NW):
                st, ln = WST[g], WCH[g]
                sync.dma_start(
                    w_sb[:, st : st + ln, :],
                    w[st * P : (st + ln) * P, :].rearrange(
                        "(c p) d -> p c d", p=P
                    ),
                ).then_inc(wsems[g], 16)
            sync.wait_ge(act_sem, 1)
            for n in (0, 2):
                sync.dma_start(
                    out[:, n * 512 : (n + 1) * 512], out_sb[n * S : (n + 1) * S, :]
                ).then_inc(osem2, 16)
            sync.wait_ge(osem2, 32)

        @block.scalar
        def _(scalar):
            scalar.dma_start(
                mbdt_sb[:], mbdt[:].rearrange("p (k j) -> p k j", k=KC1)
            ).then_inc(csem, 16)
            scalar.dma_start(bias_sb[:], bias[:]).then_inc(csem, 16)
            scalar.dma_start(ones_sb[:], onesd[:]).then_inc(csem, 16)
            scalar.wait_ge(s2_sem, 1)
            nc.scalar.activation(
                out_sb[:], po_bank[:], mybir.ActivationFunctionType.Tanh
            ).then_inc(act_sem, 1)
            scalar.wait_ge(act_sem, 1)
            for n in (1, 3):
                scalar.dma_start(
                    out[:, n * 512 : (n + 1) * 512], out_sb[n * S : (n + 1) * S, :]
                ).then_inc(osem, 16)
            scalar.wait_ge(osem, 32)

        @block.vector
        def _(vector):
            vector.wait_ge(s1_sem, 1)
            nc.vector.tensor_copy(msc_sb[:], pm_bank[:])
            nc.vector.drain()
            last = None
            for n in range(NT):
                for q in range(512 // 32):
                    d0 = n * 512 + q * 32
                    k2, r = divmod(d0, P)
                    last = nc.vector.transpose(
                        msT_sb[r : r + 32, k2, :],
                        msc_sb[n * S : (n + 1) * S, q * 32 : (q + 1) * 32],
                    )
            last.then_inc(tr_sem, 1)

        @block.tensor
        def _(tensor):
            tensor.wait_ge(csem, 48)  # mbdt/bias/ones resident
            # bias as the FIRST accumulation into po_bank (off the tail path)
            for n in range(NT):
                nc.tensor.matmul(
                    po_bank[n * S : (n + 1) * S, :],
                    ones_sb[:],
                    bias_sb[:, n * 512 : (n + 1) * 512],
                    start=True,
                    stop=False,
                    tile_position=(0, n * S),
                    skip_group_check=True,
                )
            last = None
            for g in range(NF):
                tensor.wait_ge(fsems[g], 16)
                for c in range(cf):
                    k = g * cf + c
                    for n in range(NT):
                        last = nc.tensor.matmul(
                            pm_bank[n * S : (n + 1) * S, :],
                            mbdt_sb[:, k, :],
                            feat_sb[:, k, n * 512 : (n + 1) * 512],
                            start=(k == 0),
                            stop=(k == KC1 - 1),
                            tile_position=(0, n * S),
                            skip_group_check=True,
                        )
            last.then_inc(s1_sem, 1)
            tensor.wait_ge(tr_sem, 1)
            lastb = None
            for g in range(NW):
                tensor.wait_ge(wsems[g], 16)
                for c in range(WCH[g]):
                    k2 = WST[g] + c
                    for n in range(NT):
                        lastb = nc.tensor.matmul(
                            po_bank[n * S : (n + 1) * S, :],
                            msT_sb[:, k2, :],
                            w_sb[:, k2, n * 512 : (n + 1) * 512],
                            start=False,
                            stop=(k2 == KC2 - 1),
                            tile_position=(0, n * S),
                            skip_group_check=True,
                        )
            lastb.then_inc(s2_sem, 1)

    nc.compile()
    return nc


def _host_prep(inputs: dict, compute_dtype: str):
    np_cdt = {"f32": np.float32, "bf16": None, "fp16": np.float16}[compute_dtype]
    if np_cdt is None:
        import ml_dtypes

        np_cdt = ml_dtypes.bfloat16

    feat = np.ascontiguousarray(inputs["attribute_feat"], dtype=np.float32)
    label = np.asarray(inputs["attribute_label"])
    w = np.asarray(inputs["W"], dtype=np.float32).astype(np_cdt)
    b = np.asarray(inputs["b"], dtype=np.float32).reshape(1, D).astype(np_cdt)
    mask = (label > 0).astype(np.float32)

    in_maps = []
    for c in range(NCORES):
        feat_c = feat[c * S : (c + 1) * S].reshape(S * A, D).astype(np_cdt)
        m_c = mask[c * S : (c + 1) * S]  # [S, A]
        mbd = np.zeros((KC1, P, S), np.float32)
        for k in range(KC1):
            for sl in range(P // A):  # 4 samples per 128-row chunk
                s = (P // A) * k + sl
                mbd[k, sl * A : (sl + 1) * A, s] = m_c[s]
        # device layout: [partition, (k_chunk, sample)] contiguous
        mbd_dev = np.ascontiguousarray(mbd.transpose(1, 0, 2)).reshape(P, KC1 * S)
        in_maps.append(
            {
                "feat": feat_c,
                "mbdt": mbd_dev.astype(np_cdt),
                "w": w,
                "bias": b,
                "ones": np.ones((1, S), np_cdt),
            }
        )
    return in_maps


_NC_CACHE: dict = {}


def run(inputs: dict, compute_dtype: str = COMPUTE_DTYPE, trace: bool = False):
    from concourse.bass_utils import run_bass_kernel_spmd

    impl = os.environ.get("GNN_KERNEL_IMPL", "raw")
    key = (compute_dtype, impl)
    if key not in _NC_CACHE:
        builder = build_nc_raw if impl == "raw" else build_nc
        _NC_CACHE[key] = builder(compute_dtype)
    nc = _NC_CACHE[key]
    in_maps = _host_prep(inputs, compute_dtype)
    res = run_bass_kernel_spmd(nc, in_maps, list(range(NCORES)), trace=trace)
    out = np.concatenate([res.results[c]["out"] for c in range(NCORES)], axis=0)
    return out, res


def kernel(**inputs) -> np.ndarray:
    out, _ = run(inputs)
    return out



# revision 2
# speedup vs baseline: 1.8745x; 1.8745x over previous
"""Trainium2 Bass kernel for nn_Meta_Graph1_40114994545303 (gnn_message_passing).

Math: the reference returns only the global-node row of the GCN output.
With mask = (attribute_label > 0), star adjacency means
    out[s, :] = tanh( (sum_a mask[s,a] * attribute_feat[s,a,:]) @ W + b )
and x never reaches the output (adj[A, A] = 0).

Strategy: data-parallel over batch, 32 samples per core on 8 cores.
Collectives on this runtime cost ~90us for even a 128KB AllGather (measured),
so W stays replicated and the kernel is a pure HBM-bandwidth play:
  - host relayouts feat and W into partition-major blocks so every DMA is
    128 descriptors x 16KB contiguous per partition (full line rate, no ramp)
  - feat (4MB fp16) streams first at full rate, W (8MB fp16) behind it on the
    same FIFO HWDGE queue; stage-2 matmuls chase the W stream
  - stage 1: masked sum as block-diagonal matmul; DVE 32x32 transposes to get
    the stationary operand for stage 2 (hidden under the W stream)
  - stage 2: [32,2048] @ W with bias folded in as a rank-1 matmul done first,
    tanh on the scalar engine, DMA out.
"""

import numpy as np

import concourse.bacc as bacc
import concourse.mybir as mybir

B, A, D = 256, 32, 2048
NCORES = 8
S = B // NCORES  # 32 samples per core
P = 128
KC1 = (S * A) // P  # 8 k-chunks in stage 1 (contraction over (sample, attr))
KC2 = D // P  # 16 k-chunks in stage 2 (contraction over d_in)
NT = D // 512  # 4 psum-bank-wide column tiles
F32 = mybir.dt.float32
F16 = mybir.dt.float16

NF = 2  # feat DMA groups (4 k-chunks each)
NW = 4  # W DMA groups (4 k2-chunks each)


def build_nc():
    cdt = F16
    nc = bacc.Bacc("TRN2", target_bir_lowering=False, debug=False)

    # partition-major layouts: [p, c*D] with per-partition contiguous chunks
    featd = nc.dram_tensor("feat", [P, KC1 * D], cdt, kind="ExternalInput")
    wd = nc.dram_tensor("w", [P, KC2 * D], cdt, kind="ExternalInput")
    mbdt = nc.dram_tensor("mbdt", [P, KC1 * S], cdt, kind="ExternalInput")
    bias = nc.dram_tensor("bias", [1, D], cdt, kind="ExternalInput")
    onesd = nc.dram_tensor("ones", [1, S], cdt, kind="ExternalInput")
    out = nc.dram_tensor("out", [S, D], F32, kind="ExternalOutput")

    from contextlib import ExitStack

    with ExitStack() as ctx:
        feat_sb = ctx.enter_context(nc.sbuf_tensor([P, KC1, D], cdt))
        w_sb = ctx.enter_context(nc.sbuf_tensor([P, KC2, D], cdt))
        mbdt_sb = ctx.enter_context(nc.sbuf_tensor([P, KC1, S], cdt))
        bias_sb = ctx.enter_context(nc.sbuf_tensor([1, D], cdt))
        ones_sb = ctx.enter_context(nc.sbuf_tensor([1, S], cdt))
        msc_sb = ctx.enter_context(nc.sbuf_tensor([P, 512], cdt))
        msT_sb = ctx.enter_context(nc.sbuf_tensor([P, KC2, S], cdt))
        out_sb = ctx.enter_context(nc.sbuf_tensor([P, 512], F32))
        pm_bank = ctx.enter_context(nc.psum_tensor([P, 512], F32))
        po_bank = ctx.enter_context(nc.psum_tensor([P, 512], F32))
        fsems = [ctx.enter_context(nc.semaphore(f"fs{g}")) for g in range(NF)]
        wsems = [ctx.enter_context(nc.semaphore(f"ws{g}")) for g in range(NW)]
        csem = ctx.enter_context(nc.semaphore("csem"))
        s1_sem = ctx.enter_context(nc.semaphore("s1_sem"))
        tr_sem = ctx.enter_context(nc.semaphore("tr_sem"))
        s2_sem = ctx.enter_context(nc.semaphore("s2_sem"))
        act_sem = ctx.enter_context(nc.semaphore("act_sem"))
        osem = ctx.enter_context(nc.semaphore("osem"))
        block = ctx.enter_context(nc.Block(no_gpsimd_drain=True))

        CF = KC1 // NF  # k-chunks per feat DMA
        CW = KC2 // NW  # k2-chunks per W DMA

        @block.sync
        def _(sync):
            # feat first at full rate, then W, all on one FIFO HWDGE queue;
            # per-partition contiguous (CF*D*2 = 16KB runs)
            for g in range(NF):
                sync.dma_start(
                    feat_sb[:, g * CF : (g + 1) * CF, :],
                    featd[:, g * CF * D : (g + 1) * CF * D].rearrange(
                        "p (c d) -> p c d", d=D
                    ),
                ).then_inc(fsems[g], 16)
            for g in range(NW):
                sync.dma_start(
                    w_sb[:, g * CW : (g + 1) * CW, :],
                    wd[:, g * CW * D : (g + 1) * CW * D].rearrange(
                        "p (c d) -> p c d", d=D
                    ),
                ).then_inc(wsems[g], 16)
            sync.wait_ge(act_sem, 1)
            for n in (0, 2):
                sync.dma_start(
                    out[:, n * 512 : (n + 1) * 512], out_sb[n * S : (n + 1) * S, :]
                ).then_inc(osem, 16)
            sync.wait_ge(osem, 32)

        @block.scalar
        def _(scalar):
            scalar.dma_start(
                mbdt_sb[:], mbdt[:].rearrange("p (k j) -> p k j", k=KC1)
            ).then_inc(csem, 16)
            scalar.dma_start(bias_sb[:], bias[:]).then_inc(csem, 16)
            scalar.dma_start(ones_sb[:], onesd[:]).then_inc(csem, 16)
            scalar.wait_ge(s2_sem, 1)
            nc.scalar.activation(
                out_sb[:], po_bank[:], mybir.ActivationFunctionType.Tanh
            ).then_inc(act_sem, 1)
            scalar.wait_ge(act_sem, 1)
            for n in (1, 3):
                scalar.dma_start(
                    out[:, n * 512 : (n + 1) * 512], out_sb[n * S : (n + 1) * S, :]
                ).then_inc(osem, 16)
            scalar.wait_ge(osem, 32)

        @block.vector
        def _(vector):
            vector.wait_ge(s1_sem, 1)
            nc.vector.tensor_copy(msc_sb[:], pm_bank[:])
            nc.vector.drain()
            last = None
            for n in range(NT):
                for q in range(512 // 32):
                    d0 = n * 512 + q * 32
                    k2, r = divmod(d0, P)
                    last = nc.vector.transpose(
                        msT_sb[r : r + 32, k2, :],
                        msc_sb[n * S : (n + 1) * S, q * 32 : (q + 1) * 32],
                    )
            last.then_inc(tr_sem, 1)

        @block.tensor
        def _(tensor):
            tensor.wait_ge(csem, 48)  # mbdt/bias/ones resident
            # bias as the FIRST accumulation into po_bank (off the tail path)
            for n in range(NT):
                nc.tensor.matmul(
                    po_bank[n * S : (n + 1) * S, :],
                    ones_sb[:],
                    bias_sb[:, n * 512 : (n + 1) * 512],
                    start=True,
                    stop=False,
                    tile_position=(0, n * S),
                    skip_group_check=True,
                )
            last = None
            for g in range(NF):
                tensor.wait_ge(fsems[g], 16)
                for c in range(CF):
                    k = g * CF + c
                    for n in range(NT):
                        last = nc.tensor.matmul(
                            pm_bank[n * S : (n + 1) * S, :],
                            mbdt_sb[:, k, :],
                            feat_sb[:, k, n * 512 : (n + 1) * 512],
                            start=(k == 0),
                            stop=(k == KC1 - 1),
                            tile_position=(0, n * S),
                            skip_group_check=True,
                        )
            last.then_inc(s1_sem, 1)
            tensor.wait_ge(tr_sem, 1)
            lastb = None
            for g in range(NW):
                tensor.wait_ge(wsems[g], 16)
                for c in range(CW):
                    k2 = g * CW + c
                    for n in range(NT):
                        lastb = nc.tensor.matmul(
                            po_bank[n * S : (n + 1) * S, :],
                            msT_sb[:, k2, :],
                            w_sb[:, k2, n * 512 : (n + 1) * 512],
                            start=False,
                            stop=(k2 == KC2 - 1),
                            tile_position=(0, n * S),
                            skip_group_check=True,
                        )
            lastb.then_inc(s2_sem, 1)

    nc.compile()
    return nc


def _pm(x, nchunks):
    """[nchunks*128, D] row-major -> partition-major [128, nchunks*D]."""
    d = x.shape[1]
    return np.ascontiguousarray(
        x.reshape(nchunks, P, d).transpose(1, 0, 2).reshape(P, nchunks * d)
    )


def _host_prep(inputs: dict):
    feat = np.asarray(inputs["attribute_feat"], dtype=np.float32)
    label = np.asarray(inputs["attribute_label"])
    w = np.asarray(inputs["W"], dtype=np.float32).astype(np.float16)
    b = np.asarray(inputs["b"], dtype=np.float32).reshape(1, D).astype(np.float16)
    mask = (label > 0).astype(np.float32)

    w_pm = _pm(w, KC2)
    ones = np.ones((1, S), np.float16)

    in_maps = []
    for c in range(NCORES):
        feat_c = feat[c * S : (c + 1) * S].reshape(S * A, D).astype(np.float16)
        m_c = mask[c * S : (c + 1) * S]  # [S, A]
        mbd = np.zeros((KC1, P, S), np.float32)
        for k in range(KC1):
            for sl in range(P // A):  # 4 samples per 128-row chunk
                s = (P // A) * k + sl
                mbd[k, sl * A : (sl + 1) * A, s] = m_c[s]
        mbd_dev = np.ascontiguousarray(mbd.transpose(1, 0, 2)).reshape(P, KC1 * S)
        in_maps.append(
            {
                "feat": _pm(feat_c, KC1),
                "mbdt": mbd_dev.astype(np.float16),
                "w": w_pm,
                "bias": b,
                "ones": ones,
            }
        )
    return in_maps


_NC_CACHE: dict = {}


def run(inputs: dict, trace: bool = False):
    from concourse.bass_utils import run_bass_kernel_spmd

    if "nc" not in _NC_CACHE:
        _NC_CACHE["nc"] = build_nc()
    nc = _NC_CACHE["nc"]
    in_maps = _host_prep(inputs)
    res = run_bass_kernel_spmd(nc, in_maps, list(range(NCORES)), trace=trace)
    out = np.concatenate([res.results[c]["out"] for c in range(NCORES)], axis=0)
    return out, res


def kernel(**inputs) -> np.ndarray:
    out, _ = run(inputs)
    return out


# revision 37
# speedup vs baseline: 1.9093x; 1.0186x over previous
"""Trainium2 Bass kernel for nn_Meta_Graph1_40114994545303 (gnn_message_passing).

Math: the reference returns only the global-node row of the GCN output.
With mask = (attribute_label > 0), star adjacency means
    out[s, :] = tanh( (sum_a mask[s,a] * attribute_feat[s,a,:]) @ W + b )
and x never reaches the output (adj[A, A] = 0). Data-parallel over batch,
32 samples per core on 8 cores; the kernel is HBM-bandwidth-bound, so:

- Dead-input elimination: rows with mask 0 have structurally-zero
  coefficients in the adjacency operand (same as x, which is never shipped),
  so the host stages only the live feat rows (~2.3MB vs 4MB), zero-padded to
  a 128-row-chunk multiple, partition-major so the DMA moves 16KB-contiguous
  runs per partition at line rate.
- Stage 1: masked sum as block-diagonal matmul (mask stationary, feat
  moving, four 512-col tiles packed in one PSUM bank via tile_position);
  DVE 32x32 block transposes (batched 4 blocks/instruction) build the
  stage-2 stationary, hidden under the W stream.
- Stage 2 chases the W stream k2-major; W replicated per core (collectives
  measure ~90us for even a 128KB AllGather on this runtime -- not viable).
- Bias folded in as a rank-1 matmul accumulated first (off the tail path);
  tanh + output DMA pipeline per 512-column tile.
"""

import numpy as np

import concourse.bacc as bacc
import concourse.mybir as mybir

B, A, D = 256, 32, 2048
NCORES = 8
S = B // NCORES  # 32 samples per core
P = 128
KC2 = D // P  # 16 k-chunks in stage 2 (contraction over d_in)
NT = D // 512  # 4 psum-bank-wide column tiles
F32 = mybir.dt.float32
F16 = mybir.dt.float16
I32 = mybir.dt.int32

WCH = [4, 4, 4, 3, 1]  # W DMA group sizes (k2-chunks); small last group so the
WST = [0, 4, 8, 12, 15]  # final k-chunk's matmuls start right at stream end
NW = len(WCH)


def build_nc(nch: int):
    cdt = F16
    nc = bacc.Bacc("TRN2", target_bir_lowering=False, debug=False)

    featd = nc.dram_tensor("feat", [P, nch * D], cdt, kind="ExternalInput")
    wd = nc.dram_tensor("w", [P, KC2 * D], cdt, kind="ExternalInput")
    mbdt = nc.dram_tensor("mbdt", [P, nch * S], cdt, kind="ExternalInput")
    bias = nc.dram_tensor("bias", [1, D], cdt, kind="ExternalInput")
    onesd = nc.dram_tensor("ones", [1, S], cdt, kind="ExternalInput")
    out = nc.dram_tensor("out", [S, D], F32, kind="ExternalOutput")

    from contextlib import ExitStack

    with ExitStack() as ctx:
        feat_sb = ctx.enter_context(nc.sbuf_tensor([P, nch, D], cdt))
        w_sb = ctx.enter_context(nc.sbuf_tensor([P, KC2, D], cdt))
        mbdt_sb = ctx.enter_context(nc.sbuf_tensor([P, nch, S], cdt))
        bias_sb = ctx.enter_context(nc.sbuf_tensor([1, D], cdt))
        ones_sb = ctx.enter_context(nc.sbuf_tensor([1, S], cdt))
        msc_sb = ctx.enter_context(nc.sbuf_tensor([P, 512], cdt))
        msT_sb = ctx.enter_context(nc.sbuf_tensor([P, KC2, S], cdt))
        out_sb = ctx.enter_context(nc.sbuf_tensor([P, 512], F32))
        pm_bank = ctx.enter_context(nc.psum_tensor([P, 512], F32))
        po_bank = ctx.enter_context(nc.psum_tensor([P, 512], F32))
        fsems = [ctx.enter_context(nc.semaphore(f"fs{g}")) for g in range(2)]
        wsems = [ctx.enter_context(nc.semaphore(f"ws{g}")) for g in range(NW)]
        csem = ctx.enter_context(nc.semaphore("csem"))
        s1_sem = ctx.enter_context(nc.semaphore("s1_sem"))
        tr_sem = ctx.enter_context(nc.semaphore("tr_sem"))
        s2_sem = ctx.enter_context(nc.semaphore("s2_sem"))
        act_sem = ctx.enter_context(nc.semaphore("act_sem"))
        osem = ctx.enter_context(nc.semaphore("osem"))
        block = ctx.enter_context(nc.Block(no_gpsimd_drain=True))

        # feat DMA split points (chunk counts per group)
        FS = [(0, (nch + 1) // 2), ((nch + 1) // 2, nch)]

        @block.sync
        def _(sync):
            # tiny consts first (their completion is cheap only while the
            # engines' queues are empty), then feat, then the W stream
            sync.dma_start(
                mbdt_sb[:], mbdt[:].rearrange("p (k j) -> p k j", k=nch)
            ).then_inc(csem, 16)
            sync.dma_start(bias_sb[:], bias[:]).then_inc(csem, 16)
            sync.dma_start(ones_sb[:], onesd[:]).then_inc(csem, 16)
            for g, (a0, a1) in enumerate(FS):
                sync.dma_start(
                    feat_sb[:, a0:a1, :],
                    featd[:, a0 * D : a1 * D].rearrange("p (c d) -> p c d", d=D),
                ).then_inc(fsems[g], 16)
            for g in range(NW):
                st, ln = WST[g], WCH[g]
                sync.dma_start(
                    w_sb[:, st : st + ln, :],
                    wd[:, st * D : (st + ln) * D].rearrange(
                        "p (c d) -> p c d", d=D
                    ),
                ).then_inc(wsems[g], 16)
            sync.wait_ge(act_sem, 1)
            for n in (0, 2):
                sync.dma_start(
                    out[:, n * 512 : (n + 1) * 512], out_sb[n * S : (n + 1) * S, :]
                ).then_inc(osem, 16)
            sync.wait_ge(osem, 32)

        @block.scalar
        def _(scalar):
            scalar.wait_ge(s2_sem, NT)
            nc.scalar.activation(
                out_sb[:], po_bank[:], mybir.ActivationFunctionType.Tanh
            ).then_inc(act_sem, 1)
            scalar.wait_ge(act_sem, 1)
            for n in (1, 3):
                scalar.dma_start(
                    out[:, n * 512 : (n + 1) * 512], out_sb[n * S : (n + 1) * S, :]
                ).then_inc(osem, 16)
            scalar.wait_ge(osem, 32)

        @block.vector
        def _(vector):
            # s (stage-1 psum) -> fp16, then 32x32 block transposes into the
            # stage-2 stationary; 4 strided blocks per DVE instruction
            vector.wait_ge(s1_sem, 1)
            nc.vector.tensor_copy(msc_sb[:], pm_bank[:])
            nc.vector.drain()
            lastt = None
            for n in range(NT):
                for q in range(512 // 32):
                    d0 = n * 512 + q * 32
                    k2, r = divmod(d0, P)
                    lastt = nc.vector.transpose(
                        msT_sb[r : r + 32, k2, :],
                        msc_sb[n * S : (n + 1) * S, q * 32 : (q + 1) * 32],
                    )
            lastt.then_inc(tr_sem, 1)

        @block.tensor
        def _(tensor):
            tensor.wait_ge(csem, 48)  # mbdt/bias/ones resident
            # bias as the FIRST accumulation into po_bank (off the tail path)
            for n in range(NT):
                nc.tensor.matmul(
                    po_bank[n * S : (n + 1) * S, :],
                    ones_sb[:],
                    bias_sb[:, n * 512 : (n + 1) * 512],
                    start=True,
                    stop=False,
                    tile_position=(0, n * S),
                    skip_group_check=True,
                )
            # stage 1: s[j, d] = sum_slot mbd[slot, j] * feat[slot, d]
            # (mask stationary, feat moving; 4 column tiles packed into one
            # PSUM bank at partition offsets 0/32/64/96)
            last = None
            for k in range(nch):
                if k == FS[0][0]:
                    tensor.wait_ge(fsems[0], 16)
                elif k == FS[1][0]:
                    tensor.wait_ge(fsems[1], 16)
                for n in range(NT):
                    last = nc.tensor.matmul(
                        pm_bank[n * S : (n + 1) * S, :],
                        mbdt_sb[:, k, :],
                        feat_sb[:, k, n * 512 : (n + 1) * 512],
                        start=(k == 0),
                        stop=(k == nch - 1),
                        tile_position=(0, n * S),
                        skip_group_check=True,
                    )
            last.then_inc(s1_sem, 1)
            tensor.wait_ge(tr_sem, 1)
            # stage 2 k2-major so the PE chases the W stream; at the final
            # k-chunk each column tile signals s2 so tanh/output pipeline
            for g in range(NW):
                tensor.wait_ge(wsems[g], 16)
                for c in range(WCH[g]):
                    k2 = WST[g] + c
                    for n in range(NT):
                        mm = nc.tensor.matmul(
                            po_bank[n * S : (n + 1) * S, :],
                            msT_sb[:, k2, :],
                            w_sb[:, k2, n * 512 : (n + 1) * 512],
                            start=False,
                            stop=(k2 == KC2 - 1),
                            tile_position=(0, n * S),
                            skip_group_check=True,
                        )
                        if k2 == KC2 - 1:
                            mm.then_inc(s2_sem, 1)

    nc.compile()
    return nc


def _pm(x, nchunks):
    d = x.shape[1]
    return np.ascontiguousarray(
        x.reshape(nchunks, P, d).transpose(1, 0, 2).reshape(P, nchunks * d)
    )


def _host_prep(inputs: dict):
    feat = np.asarray(inputs["attribute_feat"], dtype=np.float32)
    label = np.asarray(inputs["attribute_label"])
    w = np.asarray(inputs["W"], dtype=np.float32).astype(np.float16)
    b = np.asarray(inputs["b"], dtype=np.float32).reshape(1, D).astype(np.float16)
    mask = label > 0

    w_pm = _pm(w, KC2)
    ones = np.ones((1, S), np.float16)

    rows_per_core = [
        np.nonzero(mask[c * S : (c + 1) * S].reshape(-1))[0] for c in range(NCORES)
    ]
    nch = max(1, int(np.ceil(max(len(r) for r in rows_per_core) / P)))

    in_maps = []
    for c in range(NCORES):
        rows = rows_per_core[c]
        nslot = nch * P
        feat_c = feat[c * S : (c + 1) * S].reshape(S * A, D)
        feat_cmp = np.zeros((nslot, D), np.float16)
        feat_cmp[: len(rows)] = feat_c[rows].astype(np.float16)
        mbd = np.zeros((nch, P, S), np.float32)
        for i, r in enumerate(rows):
            mbd[i // P, i % P, r // A] = 1.0
        in_maps.append(
            {
                "feat": _pm(feat_cmp, nch),
                "mbdt": np.ascontiguousarray(mbd.transpose(1, 0, 2))
                .reshape(P, nch * S)
                .astype(np.float16),
                "w": w_pm,
                "bias": b,
                "ones": ones,
            }
        )
    return in_maps, nch


_NC_CACHE: dict = {}


def run(inputs: dict, trace: bool = False):
    from concourse.bass_utils import run_bass_kernel_spmd

    in_maps, nch = _host_prep(inputs)
    if nch not in _NC_CACHE:
        _NC_CACHE[nch] = build_nc(nch)
    nc = _NC_CACHE[nch]
    res = run_bass_kernel_spmd(nc, in_maps, list(range(NCORES)), trace=trace)
    out = np.concatenate([res.results[c]["out"] for c in range(NCORES)], axis=0)
    return out, res


def kernel(**inputs) -> np.ndarray:
    out, _ = run(inputs)
    return out


# revision 38
# speedup vs baseline: 2.0162x; 1.0560x over previous
"""Trainium2 Bass kernel for nn_Meta_Graph1_40114994545303 (gnn_message_passing).

Math: the reference returns only the global-node row of the GCN output.
With mask = (attribute_label > 0), star adjacency means
    out[s, :] = tanh( (sum_a mask[s,a] * attribute_feat[s,a,:]) @ W + b )
and x never reaches the output (adj[A, A] = 0). Data-parallel over batch,
32 samples per core on 8 cores; the kernel is HBM-bandwidth-bound, so:

- Dead-input elimination: rows with mask 0 have structurally-zero
  coefficients in the adjacency operand (same as x, which is never shipped),
  so the host stages only the live feat rows (~2.3MB vs 4MB), zero-padded to
  a 128-row-chunk multiple, partition-major so the DMA moves 16KB-contiguous
  runs per partition at line rate.
- Stage 1: masked sum as block-diagonal matmul (mask stationary, feat
  moving, four 512-col tiles packed in one PSUM bank via tile_position);
  DVE 32x32 block transposes (batched 4 blocks/instruction) build the
  stage-2 stationary, hidden under the W stream.
- Stage 2 chases the W stream k2-major; W replicated per core (collectives
  measure ~90us for even a 128KB AllGather on this runtime -- not viable).
- Bias folded in as a rank-1 matmul accumulated first (off the tail path);
  tanh + output DMA pipeline per 512-column tile.
"""

import numpy as np

import concourse.bacc as bacc
import concourse.mybir as mybir

B, A, D = 256, 32, 2048
NCORES = 8
S = B // NCORES  # 32 samples per core
P = 128
KC2 = D // P  # 16 k-chunks in stage 2 (contraction over d_in)
NT = D // 512  # 4 psum-bank-wide column tiles
F32 = mybir.dt.float32
F16 = mybir.dt.float16
I32 = mybir.dt.int32

WCH = [4, 4, 4, 3, 1]  # W DMA group sizes (k2-chunks); small last group so the
WST = [0, 4, 8, 12, 15]  # final k-chunk's matmuls start right at stream end
NW = len(WCH)


def build_nc(nch: int):
    cdt = F16
    nc = bacc.Bacc("TRN2", target_bir_lowering=False, debug=False)

    featd = nc.dram_tensor("feat", [P, nch * D], cdt, kind="ExternalInput")
    wd = nc.dram_tensor("w", [P, KC2 * D], cdt, kind="ExternalInput")
    mbdt = nc.dram_tensor("mbdt", [P, nch * S], cdt, kind="ExternalInput")
    bias = nc.dram_tensor("bias", [1, D], cdt, kind="ExternalInput")
    onesd = nc.dram_tensor("ones", [1, S], cdt, kind="ExternalInput")
    out = nc.dram_tensor("out", [S, D], F32, kind="ExternalOutput")

    from contextlib import ExitStack

    with ExitStack() as ctx:
        feat_sb = ctx.enter_context(nc.sbuf_tensor([P, nch, D], cdt))
        w_sb = ctx.enter_context(nc.sbuf_tensor([P, KC2, D], cdt))
        mbdt_sb = ctx.enter_context(nc.sbuf_tensor([P, nch, S], cdt))
        bias_sb = ctx.enter_context(nc.sbuf_tensor([1, D], cdt))
        ones_sb = ctx.enter_context(nc.sbuf_tensor([1, S], cdt))
        msc_sb = ctx.enter_context(nc.sbuf_tensor([P, 512], cdt))
        msT_sb = ctx.enter_context(nc.sbuf_tensor([P, KC2, S], cdt))
        out_sb = ctx.enter_context(nc.sbuf_tensor([P, 512], F32))
        pm_bank = ctx.enter_context(nc.psum_tensor([P, 512], F32))
        po_bank = ctx.enter_context(nc.psum_tensor([P, 512], F32))
        fsems = [ctx.enter_context(nc.semaphore(f"fs{g}")) for g in range(2)]
        wsems = [ctx.enter_context(nc.semaphore(f"ws{g}")) for g in range(NW)]
        csem = ctx.enter_context(nc.semaphore("csem"))
        s1_sem = ctx.enter_context(nc.semaphore("s1_sem"))
        tr_sem = ctx.enter_context(nc.semaphore("tr_sem"))
        s2_sem = ctx.enter_context(nc.semaphore("s2_sem"))
        act_sem = ctx.enter_context(nc.semaphore("act_sem"))
        osem = ctx.enter_context(nc.semaphore("osem"))
        block = ctx.enter_context(nc.Block(no_gpsimd_drain=True))

        # feat DMA split points (chunk counts per group)
        FS = [(0, (nch + 1) // 2), ((nch + 1) // 2, nch)]

        @block.sync
        def _(sync):
            # tiny consts first (their completion is cheap only while the
            # engines' queues are empty), then feat, then the W stream
            sync.dma_start(
                mbdt_sb[:], mbdt[:].rearrange("p (k j) -> p k j", k=nch)
            ).then_inc(csem, 16)
            sync.dma_start(bias_sb[:], bias[:]).then_inc(csem, 16)
            sync.dma_start(ones_sb[:], onesd[:]).then_inc(csem, 16)
            for g, (a0, a1) in enumerate(FS):
                sync.dma_start(
                    feat_sb[:, a0:a1, :],
                    featd[:, a0 * D : a1 * D].rearrange("p (c d) -> p c d", d=D),
                ).then_inc(fsems[g], 16)
            for g in range(NW):
                st, ln = WST[g], WCH[g]
                sync.dma_start(
                    w_sb[:, st : st + ln, :],
                    wd[:, st * D : (st + ln) * D].rearrange(
                        "p (c d) -> p c d", d=D
                    ),
                ).then_inc(wsems[g], 16)
            sync.wait_ge(act_sem, 1)
            for n in (0, 2):
                sync.dma_start(
                    out[:, n * 512 : (n + 1) * 512], out_sb[n * S : (n + 1) * S, :]
                ).then_inc(osem, 16)
            sync.wait_ge(osem, 32)

        @block.scalar
        def _(scalar):
            scalar.wait_ge(s2_sem, NT)
            nc.scalar.activation(
                out_sb[:], po_bank[:], mybir.ActivationFunctionType.Tanh
            ).then_inc(act_sem, 1)
            scalar.wait_ge(act_sem, 1)
            for n in (1, 3):
                scalar.dma_start(
                    out[:, n * 512 : (n + 1) * 512], out_sb[n * S : (n + 1) * S, :]
                ).then_inc(osem, 16)
            scalar.wait_ge(osem, 32)

        @block.vector
        def _(vector):
            # s (stage-1 psum) -> fp16, then 32x32 block transposes into the
            # stage-2 stationary; 4 strided blocks per DVE instruction
            vector.wait_ge(s1_sem, 1)
            nc.vector.tensor_copy(msc_sb[:], pm_bank[:])
            nc.vector.drain()
            # 32x32 block transposes, 4 strided blocks per DVE instruction:
            # blocks (n, q=rg+4t) share output partition rows rg*32 and map to
            # k2 = 4n+t
            lastt = None
            for n in range(NT):
                for rg in range(NT):
                    lastt = nc.vector.transpose(
                        msT_sb[rg * S : (rg + 1) * S, 4 * n : 4 * n + 4, :],
                        msc_sb[n * S : (n + 1) * S, :]
                        .rearrange("p (c q j) -> p c q j", q=NT, j=S)[
                            :, :, rg : rg + 1, :
                        ]
                        .rearrange("p c q j -> p (c q) j"),
                    )
            lastt.then_inc(tr_sem, 1)

        @block.tensor
        def _(tensor):
            tensor.wait_ge(csem, 48)  # mbdt/bias/ones resident
            # bias as the FIRST accumulation into po_bank (off the tail path)
            for n in range(NT):
                nc.tensor.matmul(
                    po_bank[n * S : (n + 1) * S, :],
                    ones_sb[:],
                    bias_sb[:, n * 512 : (n + 1) * 512],
                    start=True,
                    stop=False,
                    tile_position=(0, n * S),
                    skip_group_check=True,
                )
            # stage 1: s[j, d] = sum_slot mbd[slot, j] * feat[slot, d]
            # (mask stationary, feat moving; 4 column tiles packed into one
            # PSUM bank at partition offsets 0/32/64/96)
            last = None
            for k in range(nch):
                if k == FS[0][0]:
                    tensor.wait_ge(fsems[0], 16)
                elif k == FS[1][0]:
                    tensor.wait_ge(fsems[1], 16)
                for n in range(NT):
                    last = nc.tensor.matmul(
                        pm_bank[n * S : (n + 1) * S, :],
                        mbdt_sb[:, k, :],
                        feat_sb[:, k, n * 512 : (n + 1) * 512],
                        start=(k == 0),
                        stop=(k == nch - 1),
                        tile_position=(0, n * S),
                        skip_group_check=True,
                    )
            last.then_inc(s1_sem, 1)
            tensor.wait_ge(tr_sem, 1)
            # stage 2 k2-major so the PE chases the W stream; at the final
            # k-chunk each column tile signals s2 so tanh/output pipeline
            for g in range(NW):
                tensor.wait_ge(wsems[g], 16)
                for c in range(WCH[g]):
                    k2 = WST[g] + c
                    for n in range(NT):
                        mm = nc.tensor.matmul(
                            po_bank[n * S : (n + 1) * S, :],
                            msT_sb[:, k2, :],
                            w_sb[:, k2, n * 512 : (n + 1) * 512],
                            start=False,
                            stop=(k2 == KC2 - 1),
                            tile_position=(0, n * S),
                            skip_group_check=True,
                        )
                        if k2 == KC2 - 1:
                            mm.then_inc(s2_sem, 1)

    nc.compile()
    return nc


def _pm(x, nchunks):
    d = x.shape[1]
    return np.ascontiguousarray(
        x.reshape(nchunks, P, d).transpose(1, 0, 2).reshape(P, nchunks * d)
    )


def _host_prep(inputs: dict):
    feat = np.asarray(inputs["attribute_feat"], dtype=np.float32)
    label = np.asarray(inputs["attribute_label"])
    w = np.asarray(inputs["W"], dtype=np.float32).astype(np.float16)
    b = np.asarray(inputs["b"], dtype=np.float32).reshape(1, D).astype(np.float16)
    mask = label > 0

    w_pm = _pm(w, KC2)
    ones = np.ones((1, S), np.float16)

    rows_per_core = [
        np.nonzero(mask[c * S : (c + 1) * S].reshape(-1))[0] for c in range(NCORES)
    ]
    nch = max(1, int(np.ceil(max(len(r) for r in rows_per_core) / P)))

    in_maps = []
    for c in range(NCORES):
        rows = rows_per_core[c]
        nslot = nch * P
        feat_c = feat[c * S : (c + 1) * S].reshape(S * A, D)
        feat_cmp = np.zeros((nslot, D), np.float16)
        feat_cmp[: len(rows)] = feat_c[rows].astype(np.float16)
        mbd = np.zeros((nch, P, S), np.float32)
        for i, r in enumerate(rows):
            mbd[i // P, i % P, r // A] = 1.0
        in_maps.append(
            {
                "feat": _pm(feat_cmp, nch),
                "mbdt": np.ascontiguousarray(mbd.transpose(1, 0, 2))
                .reshape(P, nch * S)
                .astype(np.float16),
                "w": w_pm,
                "bias": b,
                "ones": ones,
            }
        )
    return in_maps, nch


_NC_CACHE: dict = {}


def run(inputs: dict, trace: bool = False):
    from concourse.bass_utils import run_bass_kernel_spmd

    in_maps, nch = _host_prep(inputs)
    if nch not in _NC_CACHE:
        _NC_CACHE[nch] = build_nc(nch)
    nc = _NC_CACHE[nch]
    res = run_bass_kernel_spmd(nc, in_maps, list(range(NCORES)), trace=trace)
    out = np.concatenate([res.results[c]["out"] for c in range(NCORES)], axis=0)
    return out, res


def kernel(**inputs) -> np.ndarray:
    out, _ = run(inputs)
    return out


# revision 41
# speedup vs baseline: 2.0529x; 1.0182x over previous
"""Trainium2 Bass kernel for nn_Meta_Graph1_40114994545303 (gnn_message_passing).

Math: the reference returns only the global-node row of the GCN output.
With mask = (attribute_label > 0), star adjacency means
    out[s, :] = tanh( (sum_a mask[s,a] * attribute_feat[s,a,:]) @ W + b )
and x never reaches the output (adj[A, A] = 0). Data-parallel over batch,
32 samples per core on 8 cores; the kernel is HBM-bandwidth-bound, so:

- Dead-input elimination: rows with mask 0 have structurally-zero
  coefficients in the adjacency operand (same as x, which is never shipped),
  so the host stages only the live feat rows (~2.3MB vs 4MB), zero-padded to
  a 128-row-chunk multiple, partition-major so the DMA moves 16KB-contiguous
  runs per partition at line rate.
- Stage 1: masked sum as block-diagonal matmul (mask stationary, feat
  moving, four 512-col tiles packed in one PSUM bank via tile_position);
  DVE 32x32 block transposes (batched 4 blocks/instruction) build the
  stage-2 stationary, hidden under the W stream.
- Stage 2 chases the W stream k2-major; W replicated per core (collectives
  measure ~90us for even a 128KB AllGather on this runtime -- not viable).
- Bias folded in as a rank-1 matmul accumulated first (off the tail path);
  tanh + output DMA pipeline per 512-column tile.
"""

import numpy as np

import concourse.bacc as bacc
import concourse.mybir as mybir

B, A, D = 256, 32, 2048
NCORES = 8
S = B // NCORES  # 32 samples per core
P = 128
KC2 = D // P  # 16 k-chunks in stage 2 (contraction over d_in)
NT = D // 512  # 4 psum-bank-wide column tiles
F32 = mybir.dt.float32
F16 = mybir.dt.float16
I32 = mybir.dt.int32

# W stream split across both HWDGE queues: groups 0-2 (chunks 0..9) go on the
# sync queue behind feat; groups 3-4 (chunks 10..15) go on the scalar queue and
# arrive early, so stage 2's tail only waits on chunk 9.
WCH = [4, 4, 2, 4, 2]
WST = [0, 4, 8, 10, 14]
NW = len(WCH)
W_SYNC_GROUPS = (0, 1, 2)
W_SCALAR_GROUPS = (3, 4)


def build_nc(nch: int):
    cdt = F16
    nc = bacc.Bacc("TRN2", target_bir_lowering=False, debug=False)

    featd = nc.dram_tensor("feat", [P, nch * D], cdt, kind="ExternalInput")
    wd = nc.dram_tensor("w", [P, KC2 * D], cdt, kind="ExternalInput")
    mbdt = nc.dram_tensor("mbdt", [P, nch * S], cdt, kind="ExternalInput")
    bias = nc.dram_tensor("bias", [1, D], cdt, kind="ExternalInput")
    onesd = nc.dram_tensor("ones", [1, S], cdt, kind="ExternalInput")
    out = nc.dram_tensor("out", [S, D], F32, kind="ExternalOutput")

    from contextlib import ExitStack

    with ExitStack() as ctx:
        feat_sb = ctx.enter_context(nc.sbuf_tensor([P, nch, D], cdt))
        w_sb = ctx.enter_context(nc.sbuf_tensor([P, KC2, D], cdt))
        mbdt_sb = ctx.enter_context(nc.sbuf_tensor([P, nch, S], cdt))
        bias_sb = ctx.enter_context(nc.sbuf_tensor([1, D], cdt))
        ones_sb = ctx.enter_context(nc.sbuf_tensor([1, S], cdt))
        msc_sb = ctx.enter_context(nc.sbuf_tensor([P, 512], cdt))
        msT_sb = ctx.enter_context(nc.sbuf_tensor([P, KC2, S], cdt))
        out_sb = ctx.enter_context(nc.sbuf_tensor([P, 512], F32))
        pm_bank = ctx.enter_context(nc.psum_tensor([P, 512], F32))
        po_bank = ctx.enter_context(nc.psum_tensor([P, 512], F32))
        fsems = [ctx.enter_context(nc.semaphore(f"fs{g}")) for g in range(2)]
        wsems = [ctx.enter_context(nc.semaphore(f"ws{g}")) for g in range(NW)]
        csem = ctx.enter_context(nc.semaphore("csem"))
        s1_sem = ctx.enter_context(nc.semaphore("s1_sem"))
        tr_sem = ctx.enter_context(nc.semaphore("tr_sem"))
        s2_sem = ctx.enter_context(nc.semaphore("s2_sem"))
        act_sem = ctx.enter_context(nc.semaphore("act_sem"))
        osem = ctx.enter_context(nc.semaphore("osem"))
        block = ctx.enter_context(nc.Block(no_gpsimd_drain=True))

        # feat DMA split points (chunk counts per group)
        FS = [(0, (nch + 1) // 2), ((nch + 1) // 2, nch)]

        def w_dma(eng, g):
            st, ln = WST[g], WCH[g]
            eng.dma_start(
                w_sb[:, st : st + ln, :],
                wd[:, st * D : (st + ln) * D].rearrange("p (c d) -> p c d", d=D),
            ).then_inc(wsems[g], 16)

        @block.sync
        def _(sync):
            for g, (a0, a1) in enumerate(FS):
                sync.dma_start(
                    feat_sb[:, a0:a1, :],
                    featd[:, a0 * D : a1 * D].rearrange("p (c d) -> p c d", d=D),
                ).then_inc(fsems[g], 16)
            for g in W_SYNC_GROUPS:
                w_dma(sync, g)
            sync.wait_ge(act_sem, 1)
            for n in (0, 2):
                sync.dma_start(
                    out[:, n * 512 : (n + 1) * 512], out_sb[n * S : (n + 1) * S, :]
                ).then_inc(osem, 16)
            sync.wait_ge(osem, 32)

        @block.scalar
        def _(scalar):
            # tiny consts first on the otherwise-idle scalar queue, then the
            # tail of the W stream (arrives early, off the stage-2 pace)
            scalar.dma_start(
                mbdt_sb[:], mbdt[:].rearrange("p (k j) -> p k j", k=nch)
            ).then_inc(csem, 16)
            scalar.dma_start(bias_sb[:], bias[:]).then_inc(csem, 16)
            scalar.dma_start(ones_sb[:], onesd[:]).then_inc(csem, 16)
            for g in W_SCALAR_GROUPS:
                w_dma(scalar, g)
            scalar.wait_ge(s2_sem, NT)
            nc.scalar.activation(
                out_sb[:], po_bank[:], mybir.ActivationFunctionType.Tanh
            ).then_inc(act_sem, 1)
            scalar.wait_ge(act_sem, 1)
            for n in (1, 3):
                scalar.dma_start(
                    out[:, n * 512 : (n + 1) * 512], out_sb[n * S : (n + 1) * S, :]
                ).then_inc(osem, 16)
            scalar.wait_ge(osem, 32)

        @block.vector
        def _(vector):
            # s (stage-1 psum) -> fp16, then 32x32 block transposes into the
            # stage-2 stationary; 4 strided blocks per DVE instruction
            vector.wait_ge(s1_sem, 1)
            nc.vector.tensor_copy(msc_sb[:], pm_bank[:])
            nc.vector.drain()
            # 32x32 block transposes, 4 strided blocks per DVE instruction:
            # blocks (n, q=rg+4t) share output partition rows rg*32 and map to
            # k2 = 4n+t
            lastt = None
            for n in range(NT):
                for rg in range(NT):
                    lastt = nc.vector.transpose(
                        msT_sb[rg * S : (rg + 1) * S, 4 * n : 4 * n + 4, :],
                        msc_sb[n * S : (n + 1) * S, :]
                        .rearrange("p (c q j) -> p c q j", q=NT, j=S)[
                            :, :, rg : rg + 1, :
                        ]
                        .rearrange("p c q j -> p (c q) j"),
                    )
            lastt.then_inc(tr_sem, 1)

        @block.tensor
        def _(tensor):
            tensor.wait_ge(csem, 48)  # mbdt/bias/ones resident
            # bias as the FIRST accumulation into po_bank (off the tail path)
            for n in range(NT):
                nc.tensor.matmul(
                    po_bank[n * S : (n + 1) * S, :],
                    ones_sb[:],
                    bias_sb[:, n * 512 : (n + 1) * 512],
                    start=True,
                    stop=False,
                    tile_position=(0, n * S),
                    skip_group_check=True,
                )
            # stage 1: s[j, d] = sum_slot mbd[slot, j] * feat[slot, d]
            # (mask stationary, feat moving; 4 column tiles packed into one
            # PSUM bank at partition offsets 0/32/64/96)
            last = None
            for k in range(nch):
                if k == FS[0][0]:
                    tensor.wait_ge(fsems[0], 16)
                elif k == FS[1][0]:
                    tensor.wait_ge(fsems[1], 16)
                for n in range(NT):
                    last = nc.tensor.matmul(
                        pm_bank[n * S : (n + 1) * S, :],
                        mbdt_sb[:, k, :],
                        feat_sb[:, k, n * 512 : (n + 1) * 512],
                        start=(k == 0),
                        stop=(k == nch - 1),
                        tile_position=(0, n * S),
                        skip_group_check=True,
                    )
            last.then_inc(s1_sem, 1)
            tensor.wait_ge(tr_sem, 1)
            # stage 2 k2-major so the PE chases the W stream; at the final
            # k-chunk each column tile signals s2 so tanh/output pipeline
            for g in range(NW):
                tensor.wait_ge(wsems[g], 16)
                for c in range(WCH[g]):
                    k2 = WST[g] + c
                    for n in range(NT):
                        mm = nc.tensor.matmul(
                            po_bank[n * S : (n + 1) * S, :],
                            msT_sb[:, k2, :],
                            w_sb[:, k2, n * 512 : (n + 1) * 512],
                            start=False,
                            stop=(k2 == KC2 - 1),
                            tile_position=(0, n * S),
                            skip_group_check=True,
                        )
                        if k2 == KC2 - 1:
                            mm.then_inc(s2_sem, 1)

    nc.compile()
    return nc


def _pm(x, nchunks):
    d = x.shape[1]
    return np.ascontiguousarray(
        x.reshape(nchunks, P, d).transpose(1, 0, 2).reshape(P, nchunks * d)
    )


def _host_prep(inputs: dict):
    feat = np.asarray(inputs["attribute_feat"], dtype=np.float32)
    label = np.asarray(inputs["attribute_label"])
    w = np.asarray(inputs["W"], dtype=np.float32).astype(np.float16)
    b = np.asarray(inputs["b"], dtype=np.float32).reshape(1, D).astype(np.float16)
    mask = label > 0

    w_pm = _pm(w, KC2)
    ones = np.ones((1, S), np.float16)

    rows_per_core = [
        np.nonzero(mask[c * S : (c + 1) * S].reshape(-1))[0] for c in range(NCORES)
    ]
    nch = max(1, int(np.ceil(max(len(r) for r in rows_per_core) / P)))

    in_maps = []
    for c in range(NCORES):
        rows = rows_per_core[c]
        nslot = nch * P
        feat_c = feat[c * S : (c + 1) * S].reshape(S * A, D)
        feat_cmp = np.zeros((nslot, D), np.float16)
        feat_cmp[: len(rows)] = feat_c[rows].astype(np.float16)
        mbd = np.zeros((nch, P, S), np.float32)
        for i, r in enumerate(rows):
            mbd[i // P, i % P, r // A] = 1.0
        in_maps.append(
            {
                "feat": _pm(feat_cmp, nch),
                "mbdt": np.ascontiguousarray(mbd.transpose(1, 0, 2))
                .reshape(P, nch * S)
                .astype(np.float16),
                "w": w_pm,
                "bias": b,
                "ones": ones,
            }
        )
    return in_maps, nch


_NC_CACHE: dict = {}


def run(inputs: dict, trace: bool = False):
    from concourse.bass_utils import run_bass_kernel_spmd

    in_maps, nch = _host_prep(inputs)
    if nch not in _NC_CACHE:
        _NC_CACHE[nch] = build_nc(nch)
    nc = _NC_CACHE[nch]
    res = run_bass_kernel_spmd(nc, in_maps, list(range(NCORES)), trace=trace)
    out = np.concatenate([res.results[c]["out"] for c in range(NCORES)], axis=0)
    return out, res


def kernel(**inputs) -> np.ndarray:
    out, _ = run(inputs)
    return out
